# revision 2
# baseline (speedup 1.0000x reference)
"""Trainium2 Bass kernel v2 for nn_DeepHierarchicalNetwork_30803505447112.

Structural insight: the hard-gumbel gate is binary, so task after depth d is
always s^k(x) where s = splitter GRU and k = number of 'continue' decisions
so far.  Device computes t_k = s^k(x) (k=1..5), the input projections
gi_d = Wih_{f,b} @ t_d, and runs all 10 encoder chains (5 depths x fwd/bwd)
interleaved in ONE 128-step scan (two wide chain-groups: all-forward and
all-backward).  The per-depth final hiddens hf_d/hb_d and per-k pooled sums
ship to host; the 160 decision bits, selection and the final 256-dim output
are tiny host math.

Sharding: pure data-parallel over batch (4 of 32 per core), weights
replicated; no cross-core communication.

Precision: recurrent state and cell math in bf16; matmuls run fp8e4m3
DoubleRow (x64-scaled weights) with fp8 *shadow copies* of the moving
operands, so fp8 noise enters only through matmul results, not the state.
The /64 rescale rides the activation scale.  z-gate weight rows are negated
so sigmoid yields (1-z) directly.
"""
import numpy as np
import ml_dtypes

from concourse.tile import TileContext
from concourse.vector_clock import ScopedClock, VectorClock

_MAX_WAITS = 1


def _patched_drain_and_barrier(self, tick_clock, wait_clock):
    gc = tick_clock.global_clock
    n = len(gc)
    procs = [(i, gc[i]) for i in range(n) if gc[i] > 0]
    for k in range(0, len(procs), _MAX_WAITS):
        group = procs[k:k + _MAX_WAITS]
        vc = VectorClock([0] * n)
        for i, t in group:
            vc.require_at_least(i, t)
        nop = self.nc.sync.nop()
        wait_clock.add_sem_waits(nop.ins, ScopedClock({None: vc}))
    self.nc.sync.drain()
    self.nc.all_engine_barrier()
    assert self.sems is not None
    popped = self.nc._tile_sem_poison_stack.pop()
    assert popped is self._sem_poison
    self.nc.clear_and_free_semaphores(list(self.sems.allocated().values()))
    self.nc.all_engine_barrier()


TileContext._drain_and_barrier = _patched_drain_and_barrier

import bass_rust as _br
import concourse.mybir as _mybir


def split_excess_waits(nc, max_waits=1):
    """Walrus in this container accepts only one sync-wait per instruction.
    Move extras onto injected same-engine nops placed just before."""
    ctr = [0]
    for f in nc.m.functions:
        for bb in f.blocks:
            new_insts = []
            for inst in bb.instructions:
                si = inst.sync_info
                waits = list(si.on_wait) if si and si.on_wait else []
                if len(waits) > max_waits:
                    extra, keep = waits[:-max_waits], waits[-max_waits:]
                    for k in range(0, len(extra), max_waits):
                        nop = _mybir.InstNoOp(
                            name=f"I-waitsplit-{ctr[0]}", ins=[], outs=[])
                        ctr[0] += 1
                        nop.engine = inst.engine
                        nop.sync_info = _br.SyncInfo(
                            on_wait=extra[k:k + max_waits], on_update=[])
                        new_insts.append(nop)
                    inst.sync_info = _br.SyncInfo(
                        on_wait=keep, on_update=list(si.on_update or []))
                new_insts.append(inst)
            bb.instructions[:] = new_insts
    return ctr[0]


LAST_SIM_TIME = [None]


def _install_sim_time_capture():
    from concourse.bass_interp import CoreSim
    if getattr(CoreSim, "_ant_time_capture", False):
        return
    orig = CoreSim.simulate

    def patched(self, *a, **k):
        r = orig(self, *a, **k)
        try:
            LAST_SIM_TIME[0] = float(self.time)
        except Exception:
            pass
        return r
    CoreSim.simulate = patched
    CoreSim._ant_time_capture = True


_install_sim_time_capture()

import concourse.bass as bass
import concourse.mybir as mybir

FP32 = mybir.dt.float32
BF16 = mybir.dt.bfloat16
FP8E4 = mybir.dt.float8e4
AF = mybir.ActivationFunctionType
ALU = mybir.AluOpType
AX = mybir.AxisListType
DR = mybir.MatmulPerfMode.DoubleRow

H = 512
KC = 4
G3 = 1536
NB = 4          # batch per core
S = 128
R = NB * S      # rows per core (b-major: col = b*128 + s)
DEPTH = 5
ARITY = 4
W20 = DEPTH * NB
SCALE = 64.0    # weight scale for fp8


class Cfg:
    def __init__(self, fp8=True):
        self.fp8 = fp8
        self.sh_dt = FP8E4 if fp8 else BF16
        self.w_dt = FP8E4 if fp8 else BF16
        self.scale = SCALE if fp8 else 1.0


def build_kernel(nc, cfg: Cfg):
    import os
    PHASES = os.environ.get("K2_PHASES", "sp,scan").split(",")
    inv = 1.0 / cfg.scale

    # ---------------- DRAM I/O ----------------
    def din(name, cols, dt):
        return nc.dram_tensor(name, [128, cols], dt, kind="ExternalInput")

    x8_d = din("x8", KC * R, cfg.sh_dt)
    w_d = {}
    for m in ("s", "f", "b"):
        w_d[f"ih_{m}"] = din(f"wih_{m}", KC * G3, cfg.w_dt)
        w_d[f"hh_{m}"] = din(f"whh_{m}", KC * G3, cfg.w_dt)
    ident_d = din("ident", 128, BF16)
    enc_out = nc.dram_tensor("enc_out", [128, 160], BF16,
                             kind="ExternalOutput")
    pooled_out = nc.dram_tensor("pooled_out", [128, 80], FP32,
                                kind="ExternalOutput")

    with TileContext(nc) as tc:
        frees = []

        def T(name, shape, dt):
            t, fr = tc.tile(shape, dt, name=name)
            frees.append(fr)
            return t

        # ---------------- persistent SBUF ----------------
        x8 = T("x8s", [128, KC * R], cfg.sh_dt)
        w = {}
        for m in ("s", "f", "b"):
            w[f"ih_{m}"] = T(f"wih_{m}s", [128, KC * G3], cfg.w_dt)
            w[f"hh_{m}"] = T(f"whh_{m}s", [128, KC * G3], cfg.w_dt)
        ident = T("idents", [128, 128], BF16)
        ts = T("ts", [128, KC * R], BF16)
        t8 = T("t8", [128, KC * R], cfg.sh_dt)
        ht = [T(f"ht{i}", [128, KC * R], BF16) for i in range(2)]
        h8 = [T(f"h8{i}", [128, KC * R], cfg.sh_dt) for i in range(2)]
        gi_n = T("gi_n", [128, 4 * R], BF16)
        gf = T("gf", [128, S * 240], BF16)
        gb = T("gb", [128, S * 240], BF16)
        hs8 = {c: [T(f"hs8{c}{i}", [128, 80], cfg.sh_dt) for i in range(2)]
               for c in "fb"}
        pooled_sb = T("pooled_sb", [128, 80], FP32)
        enc_sb = T("enc_sb", [128, 160], BF16)

        # ---------------- load inputs ----------------
        nc.gpsimd.dma_start(w["ih_s"][:], w_d["ih_s"][:, :])
        nc.gpsimd.dma_start(x8[:], x8_d[:, :])
        nc.gpsimd.dma_start(w["hh_s"][:], w_d["hh_s"][:, :])
        nc.sync.dma_start(ident[:], ident_d[:, :])
        nc.sync.dma_start(w["ih_f"][:], w_d["ih_f"][:, :])
        nc.sync.dma_start(w["ih_b"][:], w_d["ih_b"][:, :])
        nc.scalar.dma_start(w["hh_f"][:], w_d["hh_f"][:, :])
        nc.scalar.dma_start(w["hh_b"][:], w_d["hh_b"][:, :])

        def wap(wt, j, pr):
            """lhsT ap for DoubleRow chunk-pair pr of gate-block j."""
            return wt[:].rearrange("p (c g) -> p c g", c=KC)[
                :, 2 * pr:2 * pr + 2, j * 128:(j + 1) * 128]

        def mov(src, n_src, pr):
            return src[:].rearrange("p (c x) -> p c x", c=KC)[
                :, 2 * pr:2 * pr + 2, :]

        def emit_gemm(P, wt, j, src, n_src, start=True, stop=True):
            """psum P (+)= W_j^T @ src."""
            if cfg.fp8:
                for pr in range(2):
                    nc.tensor.matmul(
                        P, wap(wt, j, pr), mov(src, n_src, pr),
                        start=start and pr == 0, stop=stop and (pr == 1),
                        perf_mode=DR, skip_group_check=True)
            else:
                for c in range(KC):
                    nc.tensor.matmul(
                        P, wt[:, c * G3 + j * 128:c * G3 + j * 128 + 128],
                        src[:, c * n_src:(c + 1) * n_src],
                        start=start and c == 0, stop=stop and (c == KC - 1),
                        skip_group_check=True)

        # ---------------- pools ----------------
        pool_box = {}

        with tc.tile_pool(name="stp", bufs=2) as stp, \
             tc.tile_pool(name="tmp", bufs=2) as tmp:

            def proj(src8, d):
                """gi_{f,b}[d] = Wih_{f,b} @ t_d, written depth-interleaved.
                psum -> gi moves ride the DMA engines, not DVE/Act."""
                for ci, chain in enumerate("fb"):
                    gt = gf if chain == "f" else gb
                    for jp in range(6):
                        P = pool_box['p'].tile([128, 1024], FP32,
                                      tag=f"c{'01' if jp % 2 == 0 else '23'}")
                        for h_ in range(2):
                            j = 2 * jp + h_
                            emit_gemm(P[:, h_ * 512:(h_ + 1) * 512],
                                      w[f"ih_{chain}"], j, src8, R)
                        j0 = 2 * jp
                        dst = gt[:].rearrange("p (s j db) -> p j s db",
                                              j=12, db=20)[
                            :, j0:j0 + 2, :, d * 4:d * 4 + 4]
                        srcv = P[:].rearrange("p (j b s) -> p j s b", j=2,
                                              b=NB)
                        if (jp + ci) % 2 == 0:
                            nc.vector.tensor_copy(dst, srcv)
                        else:
                            nc.scalar.activation(dst, srcv, AF.Copy)

            def pooled_k(src, k):
                for c in range(KC):
                    nc.vector.tensor_reduce(
                        pooled_sb[:, (k - 1) * 16 + c * 4:
                                  (k - 1) * 16 + c * 4 + 4],
                        src[:, c * R:(c + 1) * R].rearrange(
                            "p (b s) -> p b s", b=NB),
                        axis=AX.X, op=ALU.add)

            def cell_chunk(cp, pg_r, pg_z, pg_n, hcur, hnxt, h8nxt, first):
                """GRU cell for hidden chunk-pair cp (free dim = 2R)."""
                R2 = 2 * R
                sl = slice(cp * R2, (cp + 1) * R2)
                oz_sb = tmp.tile([128, R2], BF16, tag="oz_sb")
                n_sb = tmp.tile([128, R2], BF16, tag="n_sb")
                nc.scalar.activation(oz_sb[:], pg_z[:], AF.Sigmoid, scale=inv)
                if first:
                    # r1 is unused: n1 = tanh(gi_n) since h0 = 0
                    nc.scalar.activation(n_sb[:], pg_n[:], AF.Tanh, scale=inv)
                    # keep n-gate gi for steps 2-4 (still x scale)
                    if cp == 0:
                        nc.vector.tensor_copy(gi_n[:, sl], pg_n[:])
                    else:
                        nc.scalar.activation(gi_n[:, sl], pg_n[:], AF.Copy)
                    # h1 = (1-z)*n, dual output (bf16 + fp8 shadow)
                    nc.vector.tensor_tensor(hnxt[:, sl], oz_sb[:], n_sb[:],
                                            op=ALU.mult)
                    nc.gpsimd.tensor_tensor(h8nxt[:, sl], oz_sb[:], n_sb[:],
                                            op=ALU.mult)
                else:
                    r_sb = tmp.tile([128, R2], BF16, tag="r_sb")
                    nc.scalar.activation(r_sb[:], pg_r[:], AF.Sigmoid,
                                         scale=inv)
                    t1 = tmp.tile([128, R2], BF16, tag="t1")
                    nc.vector.tensor_tensor(t1[:], r_sb[:], pg_n[:],
                                            op=ALU.mult)
                    t2 = tmp.tile([128, R2], BF16, tag="t2")
                    nc.vector.tensor_tensor(t2[:], t1[:], gi_n[:, sl],
                                            op=ALU.add)
                    nc.scalar.activation(n_sb[:], t2[:], AF.Tanh, scale=inv)
                    # off-path (after sigmoid_z): m = oz*h, w = h - m
                    m_ = tmp.tile([128, R2], BF16, tag="t2")
                    nc.gpsimd.tensor_tensor(m_[:], oz_sb[:], hcur[:, sl],
                                            op=ALU.mult)
                    w_ = tmp.tile([128, R2], BF16, tag="r_sb")
                    nc.vector.tensor_tensor(w_[:], hcur[:, sl], m_[:],
                                            op=ALU.subtract)
                    # path: v = oz*n; h' = v + w (bf16 DVE, fp8 Pool)
                    v_ = tmp.tile([128, R2], BF16, tag="t1")
                    nc.vector.tensor_tensor(v_[:], oz_sb[:], n_sb[:],
                                            op=ALU.mult)
                    nc.vector.tensor_tensor(hnxt[:, sl], v_[:], w_[:],
                                            op=ALU.add)
                    nc.gpsimd.tensor_tensor(h8nxt[:, sl], v_[:], w_[:],
                                            op=ALU.add)

            def app(src8, dst, dst8, k):
                """One splitter application: t_k = s(t_{k-1})."""
                for st in range(ARITY):
                    first = st == 0
                    hcur = ht[(st + 1) % 2]
                    h8cur = src8 if first else h8[(st + 1) % 2]
                    hnxt = dst if st == ARITY - 1 else ht[st % 2]
                    h8nxt = dst8 if st == ARITY - 1 else h8[st % 2]
                    pg = {}
                    for gi_, g in enumerate("rzn"):
                        for cp in range(2):
                            if first and g == "r":
                                continue
                            P = pool_box['p'].tile([128, 1024], FP32,
                                          tag=f"c{'01' if cp == 0 else '23'}")
                            for h_ in range(2):
                                j = gi_ * 4 + 2 * cp + h_
                                Ph = P[:, h_ * 512:(h_ + 1) * 512]
                                if first:
                                    emit_gemm(Ph, w["ih_s"], j, src8, R)
                                elif g != "n":
                                    emit_gemm(Ph, w["ih_s"], j, src8, R,
                                              start=True, stop=False)
                                    emit_gemm(Ph, w["hh_s"], j, h8cur, R,
                                              start=False, stop=True)
                                else:
                                    emit_gemm(Ph, w["hh_s"], j, h8cur, R)
                            pg[(g, cp)] = P
                    for cp in range(2):
                        cell_chunk(cp, pg.get(("r", cp)), pg[("z", cp)],
                                   pg[("n", cp)], hcur, hnxt, h8nxt, first)

            # ---------------- phase S+P ----------------
            if "sp" in PHASES:
                with tc.tile_pool(name="bigp", bufs=2, space="PSUM") as bigp:
                    pool_box['p'] = bigp
                    proj(x8, 0)
                    prev8 = x8
                    for k in range(1, DEPTH):
                        app(prev8, ts, t8, k)
                        pooled_k(ts, k)
                        proj(t8, k)
                        prev8 = t8
            elif "apponly" in PHASES:
                prev8 = x8
                for k in range(1, DEPTH + 1):
                    app(prev8, ts, t8, k)
                    pooled_k(ts, k)
                    prev8 = t8
            elif "projonly" in PHASES:
                for k in range(5):
                    proj(x8, k)

        # ---------------- scan ----------------
        if True:
            for c in "fb":
                nc.vector.memset(hs8[c][0][:], 0.0)

            def scan_step(chain, t):
                gt = gf if chain == "f" else gb
                s = t if chain == "f" else S - 1 - t
                cur8 = hs8[chain][t % 2]
                nxt8 = hs8[chain][1 - t % 2]
                pt = scp.tile([128, 512], FP32, tag=chain)
                rz = pt
                nn = pt
                wt = w[f"hh_{chain}"]
                # preload gi_rz via identity-matmul (start=True zeroes bank)
                nc.tensor.matmul(rz[:, 0:160], ident[:],
                                 gt[:, s * 240:s * 240 + 160],
                                 start=True, stop=False,
                                 skip_group_check=True)
                for j in range(8):
                    o = rz[:, j * W20:(j + 1) * W20]
                    if cfg.fp8:
                        for pr in range(2):
                            nc.tensor.matmul(
                                o, wap(wt, j, pr), mov(cur8, W20, pr),
                                start=False, stop=(pr == 1),
                                perf_mode=DR, skip_group_check=True)
                    else:
                        for cc in range(KC):
                            nc.tensor.matmul(
                                o, wt[:, cc * G3 + j * 128:
                                      cc * G3 + j * 128 + 128],
                                cur8[:, cc * W20:(cc + 1) * W20],
                                start=False, stop=(cc == KC - 1),
                                skip_group_check=True)
                for j in range(8, 12):
                    o = nn[:, 160 + (j - 8) * W20:160 + (j - 7) * W20]
                    if cfg.fp8:
                        for pr in range(2):
                            nc.tensor.matmul(
                                o, wap(wt, j, pr), mov(cur8, W20, pr),
                                start=(pr == 0), stop=(pr == 1),
                                perf_mode=DR, skip_group_check=True)
                    else:
                        for cc in range(KC):
                            nc.tensor.matmul(
                                o, wt[:, cc * G3 + j * 128:
                                      cc * G3 + j * 128 + 128],
                                cur8[:, cc * W20:(cc + 1) * W20],
                                start=(cc == 0), stop=(cc == KC - 1),
                                skip_group_check=True)
                rz_sb = stp.tile([128, 160], BF16, tag=f"r{chain}")
                nc.scalar.activation(rz_sb[:], rz[:, 0:160], AF.Sigmoid,
                                     scale=inv)
                t1 = stp.tile([128, 80], BF16, tag=f"t1{chain}")
                nc.vector.tensor_tensor(t1[:], rz_sb[:, 0:80],
                                        nn[:, 160:240], op=ALU.mult)
                t2 = stp.tile([128, 80], BF16, tag=f"t2{chain}")
                nc.vector.tensor_tensor(
                    t2[:], t1[:], gt[:, s * 240 + 160:s * 240 + 240],
                    op=ALU.add)
                n_sb = stp.tile([128, 80], BF16, tag=f"n{chain}")
                nc.scalar.activation(n_sb[:], t2[:], AF.Tanh, scale=inv)
                # off-path: m = oz*h, w = h - m (pool is dtype-blind)
                m_ = stp.tile([128, 80], BF16, tag=f"m{chain}")
                nc.gpsimd.tensor_tensor(m_[:], rz_sb[:, 80:160], cur8[:],
                                        op=ALU.mult)
                w_ = stp.tile([128, 80], BF16, tag=f"w{chain}")
                nc.gpsimd.tensor_tensor(w_[:], cur8[:], m_[:],
                                        op=ALU.subtract)
                # path: v = oz*n; h'8 = v + w
                v_ = stp.tile([128, 80], BF16, tag=f"v{chain}")
                nc.vector.tensor_tensor(v_[:], rz_sb[:, 80:160], n_sb[:],
                                        op=ALU.mult)
                nc.gpsimd.tensor_tensor(nxt8[:], v_[:], w_[:], op=ALU.add)

            if "scan" in PHASES:
                for t in range(S):
                    scan_step("f", t)
                    scan_step("b", t)
            if "sp" in PHASES:
                app(t8, ts, t8, DEPTH)
                pooled_k(ts, DEPTH)

            fin = S % 2
            nc.vector.tensor_copy(enc_sb[:, 0:80], hs8["f"][fin][:])
            nc.vector.tensor_copy(enc_sb[:, 80:160], hs8["b"][fin][:])
            nc.gpsimd.dma_start(enc_out[:, :], enc_sb[:])
            nc.gpsimd.dma_start(pooled_out[:, :], pooled_sb[:])
            app5_pool.__exit__(None, None, None)
            scan_pools.__exit__(None, None, None)

        for fr in reversed(frees):
            fr()
    return nc


# ---------------- host side ----------------

def _np_dt(dt):
    return ml_dtypes.float8_e4m3 if dt == FP8E4 else ml_dtypes.bfloat16


def make_inmaps(p, cfg: Cfg):
    x = np.asarray(p["x"], np.float32)
    ins = []

    def wprep(W):
        Wm = W.astype(np.float32) * cfg.scale
        Wm = Wm.copy()
        Wm[512:1024] *= -1.0  # negate z rows -> sigmoid gives 1-z
        arr = Wm.T.reshape(KC, 128, G3).transpose(1, 0, 2).reshape(
            128, KC * G3)
        return np.ascontiguousarray(arr).astype(_np_dt(cfg.w_dt))

    wmats = {}
    for m, pref in (("s", "ts"), ("f", "tgf"), ("b", "tgb")):
        for nm in ("bih", "bhh"):
            assert not np.any(np.asarray(p[f"{pref}_{nm}"])), \
                f"nonzero bias {pref}_{nm} not supported"
        wmats[f"wih_{m}"] = wprep(np.asarray(p[f"{pref}_Wih"]))
        wmats[f"whh_{m}"] = wprep(np.asarray(p[f"{pref}_Whh"]))
    ident = np.eye(128, dtype=ml_dtypes.bfloat16)

    for c in range(8):
        m = dict(wmats)
        m["ident"] = ident
        xl = x[4 * c:4 * c + 4]           # (4, S, H)
        arr = xl.transpose(2, 0, 1).reshape(KC, 128, NB * S)
        m["x8"] = np.ascontiguousarray(
            arr.transpose(1, 0, 2).reshape(128, KC * R)).astype(
                _np_dt(cfg.sh_dt))
        ins.append(m)
    return ins


def gather_out(results, p):
    EPS = 1e-10
    x = np.asarray(p["x"], np.float64)
    g = -np.log(-np.log(np.asarray(p["gumbel_u"], np.float64) + EPS) + EPS)
    lw = np.asarray(p["logits_W"], np.float64)
    lb = np.asarray(p["logits_b"], np.float64)
    dw = lw[1] - lw[0]
    db = lb[1] - lb[0]
    out_W = np.asarray(p["out_W"], np.float64)
    out_b = np.asarray(p["out_b"], np.float64)
    B = 32

    pooled = np.zeros((DEPTH + 1, B, H))
    pooled[0] = x.mean(1)
    margins = np.zeros((DEPTH, B))
    for c, r in enumerate(results):
        enc = np.asarray(r["enc_out"], np.float64)      # (128, 160)
        pp = np.asarray(r["pooled_out"], np.float64)    # (128, 80)
        hf = enc[:, 0:80]
        hb = enc[:, 80:160]
        for bl in range(NB):
            b = 4 * c + bl
            for d in range(DEPTH):
                vec = np.zeros(H)
                for ch in range(KC):
                    col = ch * 20 + d * 4 + bl
                    vec[ch * 128:(ch + 1) * 128] = hf[:, col] + hb[:, col]
                margins[d, b] = vec @ dw + db + g[d, b, 1] - g[d, b, 0]
            for k in range(1, DEPTH + 1):
                vec = np.zeros(H)
                for ch in range(KC):
                    col = (k - 1) * 16 + ch * 4 + bl
                    vec[ch * 128:(ch + 1) * 128] = pp[:, col]
                pooled[k, b] = vec / S

    out = np.zeros(256)
    for b in range(B):
        mb = 0
        for d in range(DEPTH):
            if margins[d, b] > 0:
                mb += 1
            else:
                break
        out += pooled[mb, b] @ out_W.T
    out += B * out_b
    return out.astype(np.float32)


_BUILT = {}
PREDICTED_NS = [None]


def _get_built(cfg=None):
    key = "k"
    if key not in _BUILT:
        if cfg is None:
            cfg = Cfg()
        nc = bass.Bass(trn_type="TRN2")
        build_kernel(nc, cfg)
        split_excess_waits(nc)
        PREDICTED_NS[0] = LAST_SIM_TIME[0]
        _BUILT[key] = (nc, cfg)
    return _BUILT[key]


def kernel(**inputs):
    from concourse import bass_utils
    inputs = {k: np.asarray(v) for k, v in inputs.items()}
    nc, cfg = _get_built()
    ins = make_inmaps(inputs, cfg)
    res = bass_utils.run_bass_kernel_spmd(nc, ins, core_ids=list(range(8)))
    return gather_out(res.results, inputs)


if __name__ == "__main__":
    import sys
    cfg = Cfg(fp8="nofp8" not in sys.argv)
    _get_built(cfg)
    print(f"HW exec time: {PREDICTED_NS[0]:.0f} ns")


# revision 3
# speedup vs baseline: 1.0281x; 1.0281x over previous
"""Trainium2 Bass kernel v2 for nn_DeepHierarchicalNetwork_30803505447112.

Structural insight: the hard-gumbel gate is binary, so task after depth d is
always s^k(x) where s = splitter GRU and k = number of 'continue' decisions
so far.  Device computes t_k = s^k(x) (k=1..5), the input projections
gi_d = Wih_{f,b} @ t_d, and runs all 10 encoder chains (5 depths x fwd/bwd)
interleaved in ONE 128-step scan (two wide chain-groups: all-forward and
all-backward).  The per-depth final hiddens hf_d/hb_d and per-k pooled sums
ship to host; the 160 decision bits, selection and the final 256-dim output
are tiny host math.

Sharding: pure data-parallel over batch (4 of 32 per core), weights
replicated; no cross-core communication.

Precision: recurrent state and cell math in bf16; matmuls run fp8e4m3
DoubleRow (x64-scaled weights) with fp8 *shadow copies* of the moving
operands, so fp8 noise enters only through matmul results, not the state.
The /64 rescale rides the activation scale.  z-gate weight rows are negated
so sigmoid yields (1-z) directly.
"""
import numpy as np
import ml_dtypes

from concourse.tile import TileContext
from concourse.vector_clock import ScopedClock, VectorClock

_MAX_WAITS = 1


def _patched_drain_and_barrier(self, tick_clock, wait_clock):
    gc = tick_clock.global_clock
    n = len(gc)
    procs = [(i, gc[i]) for i in range(n) if gc[i] > 0]
    for k in range(0, len(procs), _MAX_WAITS):
        group = procs[k:k + _MAX_WAITS]
        vc = VectorClock([0] * n)
        for i, t in group:
            vc.require_at_least(i, t)
        nop = self.nc.sync.nop()
        wait_clock.add_sem_waits(nop.ins, ScopedClock({None: vc}))
    self.nc.sync.drain()
    self.nc.all_engine_barrier()
    assert self.sems is not None
    popped = self.nc._tile_sem_poison_stack.pop()
    assert popped is self._sem_poison
    self.nc.clear_and_free_semaphores(list(self.sems.allocated().values()))
    self.nc.all_engine_barrier()


TileContext._drain_and_barrier = _patched_drain_and_barrier

import bass_rust as _br
import concourse.mybir as _mybir


def split_excess_waits(nc, max_waits=1):
    """Walrus in this container accepts only one sync-wait per instruction.
    Move extras onto injected same-engine nops placed just before."""
    ctr = [0]
    for f in nc.m.functions:
        for bb in f.blocks:
            new_insts = []
            for inst in bb.instructions:
                si = inst.sync_info
                waits = list(si.on_wait) if si and si.on_wait else []
                if len(waits) > max_waits:
                    extra, keep = waits[:-max_waits], waits[-max_waits:]
                    for k in range(0, len(extra), max_waits):
                        nop = _mybir.InstNoOp(
                            name=f"I-waitsplit-{ctr[0]}", ins=[], outs=[])
                        ctr[0] += 1
                        nop.engine = inst.engine
                        nop.sync_info = _br.SyncInfo(
                            on_wait=extra[k:k + max_waits], on_update=[])
                        new_insts.append(nop)
                    inst.sync_info = _br.SyncInfo(
                        on_wait=keep, on_update=list(si.on_update or []))
                new_insts.append(inst)
            bb.instructions[:] = new_insts
    return ctr[0]


LAST_SIM_TIME = [None]


def _install_sim_time_capture():
    from concourse.bass_interp import CoreSim
    if getattr(CoreSim, "_ant_time_capture", False):
        return
    orig = CoreSim.simulate

    def patched(self, *a, **k):
        r = orig(self, *a, **k)
        try:
            LAST_SIM_TIME[0] = float(self.time)
        except Exception:
            pass
        return r
    CoreSim.simulate = patched
    CoreSim._ant_time_capture = True


_install_sim_time_capture()

import concourse.bass as bass
import concourse.mybir as mybir

FP32 = mybir.dt.float32
BF16 = mybir.dt.bfloat16
FP8E4 = mybir.dt.float8e4
AF = mybir.ActivationFunctionType
ALU = mybir.AluOpType
AX = mybir.AxisListType
DR = mybir.MatmulPerfMode.DoubleRow

H = 512
KC = 4
G3 = 1536
NB = 4          # batch per core
S = 128
R = NB * S      # rows per core (b-major: col = b*128 + s)
DEPTH = 5
ARITY = 4
W20 = DEPTH * NB
SCALE = 64.0    # weight scale for fp8


class Cfg:
    def __init__(self, fp8=True):
        self.fp8 = fp8
        self.sh_dt = FP8E4 if fp8 else BF16
        self.w_dt = FP8E4 if fp8 else BF16
        self.scale = SCALE if fp8 else 1.0


def build_kernel(nc, cfg: Cfg):
    import os
    PHASES = os.environ.get("K2_PHASES", "sp,scan").split(",")
    inv = 1.0 / cfg.scale

    # ---------------- DRAM I/O ----------------
    def din(name, cols, dt):
        return nc.dram_tensor(name, [128, cols], dt, kind="ExternalInput")

    x8_d = din("x8", KC * R, cfg.sh_dt)
    w_d = {}
    for m in ("s", "f", "b"):
        w_d[f"ih_{m}"] = din(f"wih_{m}", KC * G3, cfg.w_dt)
        w_d[f"hh_{m}"] = din(f"whh_{m}", KC * G3, cfg.w_dt)
    ident_d = din("ident", 128, BF16)
    enc_out = nc.dram_tensor("enc_out", [128, 160], BF16,
                             kind="ExternalOutput")
    pooled_out = nc.dram_tensor("pooled_out", [128, 80], FP32,
                                kind="ExternalOutput")

    with TileContext(nc) as tc:
        frees = []

        def T(name, shape, dt):
            t, fr = tc.tile(shape, dt, name=name)
            frees.append(fr)
            return t

        # ---------------- persistent SBUF ----------------
        x8 = T("x8s", [128, KC * R], cfg.sh_dt)
        w = {}
        for m in ("s", "f", "b"):
            w[f"ih_{m}"] = T(f"wih_{m}s", [128, KC * G3], cfg.w_dt)
            w[f"hh_{m}"] = T(f"whh_{m}s", [128, KC * G3], cfg.w_dt)
        ident = T("idents", [128, 128], BF16)
        ts = T("ts", [128, KC * R], BF16)
        t8 = T("t8", [128, KC * R], cfg.sh_dt)
        ht = [T(f"ht{i}", [128, KC * R], BF16) for i in range(2)]
        h8 = [T(f"h8{i}", [128, KC * R], cfg.sh_dt) for i in range(2)]
        gi_n = T("gi_n", [128, 4 * R], BF16)
        gf = T("gf", [128, S * 240], BF16)
        gb = T("gb", [128, S * 240], BF16)
        hs8 = {c: [T(f"hs8{c}{i}", [128, 80], cfg.sh_dt) for i in range(2)]
               for c in "fb"}
        pooled_sb = T("pooled_sb", [128, 80], FP32)
        enc_sb = T("enc_sb", [128, 160], BF16)

        # ---------------- load inputs ----------------
        nc.gpsimd.dma_start(w["ih_s"][:], w_d["ih_s"][:, :])
        nc.gpsimd.dma_start(x8[:], x8_d[:, :])
        nc.gpsimd.dma_start(w["hh_s"][:], w_d["hh_s"][:, :])
        nc.sync.dma_start(ident[:], ident_d[:, :])
        nc.sync.dma_start(w["ih_f"][:], w_d["ih_f"][:, :])
        nc.sync.dma_start(w["ih_b"][:], w_d["ih_b"][:, :])
        nc.scalar.dma_start(w["hh_f"][:], w_d["hh_f"][:, :])
        nc.scalar.dma_start(w["hh_b"][:], w_d["hh_b"][:, :])

        def wap(wt, j, pr):
            """lhsT ap for DoubleRow chunk-pair pr of gate-block j."""
            return wt[:].rearrange("p (c g) -> p c g", c=KC)[
                :, 2 * pr:2 * pr + 2, j * 128:(j + 1) * 128]

        def mov(src, n_src, pr, hf_=None):
            ap = src[:].rearrange("p (c x) -> p c x", c=KC)[
                :, 2 * pr:2 * pr + 2, :]
            if hf_ is None:
                return ap
            HR = R // 2
            return ap[:, :, hf_ * HR:hf_ * HR + HR]

        def emit_gemm(P, wt, j, src, n_src, start=True, stop=True, hf_=None):
            """psum P (+)= W_j^T @ src (optionally a row-half of src)."""
            for pr in range(2):
                nc.tensor.matmul(
                    P, wap(wt, j, pr), mov(src, n_src, pr, hf_),
                    start=start and pr == 0, stop=stop and (pr == 1),
                    perf_mode=DR, skip_group_check=True)

        # ---------------- pools ----------------
        pool_box = {}

        with tc.tile_pool(name="stp", bufs=2) as stp, \
             tc.tile_pool(name="tmp", bufs=2) as tmp:

            def proj(src8, d):
                """gi_{f,b}[d] = Wih_{f,b} @ t_d, written depth-interleaved.
                psum -> gi moves ride the DMA engines, not DVE/Act."""
                for ci, chain in enumerate("fb"):
                    gt = gf if chain == "f" else gb
                    for jp in range(6):
                        P = pool_box['p'].tile([128, 1024], FP32,
                                               tag=f"h{jp % 2}")
                        for h_ in range(2):
                            j = 2 * jp + h_
                            emit_gemm(P[:, h_ * 512:(h_ + 1) * 512],
                                      w[f"ih_{chain}"], j, src8, R)
                        j0 = 2 * jp
                        dst = gt[:].rearrange("p (s j db) -> p j s db",
                                              j=12, db=20)[
                            :, j0:j0 + 2, :, d * 4:d * 4 + 4]
                        srcv = P[:].rearrange("p (j b s) -> p j s b", j=2,
                                              b=NB)
                        if (jp + ci) % 2 == 0:
                            nc.vector.tensor_copy(dst, srcv)
                        else:
                            nc.scalar.activation(dst, srcv, AF.Copy)

            def pooled_k(src, k):
                for c in range(KC):
                    nc.vector.tensor_reduce(
                        pooled_sb[:, (k - 1) * 16 + c * 4:
                                  (k - 1) * 16 + c * 4 + 4],
                        src[:, c * R:(c + 1) * R].rearrange(
                            "p (b s) -> p b s", b=NB),
                        axis=AX.X, op=ALU.add)

            def cell_chunk(hf_, pg_r, pg_z, pg_n, hcur, hnxt, h8nxt,
                           first):
                """GRU cell for row-half hf_, all 4 chunks (free = 2R)."""
                R2 = 2 * R
                HR = R // 2

                def hsl(t_):
                    # 3D ap: all chunks x rows [hf_*HR, +HR)
                    return t_[:].rearrange("p (c r) -> p c r", c=KC)[
                        :, :, hf_ * HR:hf_ * HR + HR]
                gsl = gi_n[:].rearrange("p (c r) -> p c r", c=KC)[
                    :, :, hf_ * HR:hf_ * HR + HR]
                oz_sb = tmp.tile([128, R2], BF16, tag="oz_sb")
                n_sb = tmp.tile([128, R2], BF16, tag="n_sb")
                nc.scalar.activation(oz_sb[:], pg_z[:], AF.Sigmoid, scale=inv)
                o3 = lambda t_: t_[:].rearrange("p (c r) -> p c r", c=KC)
                if first:
                    # r1 is unused: n1 = tanh(gi_n) since h0 = 0
                    nc.scalar.activation(n_sb[:], pg_n[:], AF.Tanh, scale=inv)
                    # keep n-gate gi for steps 2-4 (still x scale)
                    if hf_ == 0:
                        nc.vector.tensor_copy(gsl, o3(pg_n))
                    else:
                        nc.scalar.activation(gsl, o3(pg_n), AF.Copy)
                    # h1 = (1-z)*n, dual output (bf16 + fp8 shadow)
                    nc.vector.tensor_tensor(hsl(hnxt), o3(oz_sb), o3(n_sb),
                                            op=ALU.mult)
                    nc.gpsimd.tensor_tensor(hsl(h8nxt), o3(oz_sb), o3(n_sb),
                                            op=ALU.mult)
                else:
                    r_sb = tmp.tile([128, R2], BF16, tag="r_sb")
                    nc.scalar.activation(r_sb[:], pg_r[:], AF.Sigmoid,
                                         scale=inv)
                    t1 = tmp.tile([128, R2], BF16, tag="t1")
                    nc.vector.tensor_tensor(t1[:], r_sb[:], pg_n[:],
                                            op=ALU.mult)
                    t2 = tmp.tile([128, R2], BF16, tag="t2")
                    nc.vector.tensor_tensor(o3(t2), o3(t1), gsl, op=ALU.add)
                    nc.scalar.activation(n_sb[:], t2[:], AF.Tanh, scale=inv)
                    # off-path (after sigmoid_z): m = oz*h, w = h - m
                    m_ = tmp.tile([128, R2], BF16, tag="t2")
                    nc.gpsimd.tensor_tensor(o3(m_), o3(oz_sb), hsl(hcur),
                                            op=ALU.mult)
                    w_ = tmp.tile([128, R2], BF16, tag="r_sb")
                    nc.gpsimd.tensor_tensor(o3(w_), hsl(hcur), o3(m_),
                                            op=ALU.subtract)
                    # path: v = oz*n; h' = v + w (bf16 DVE, fp8 Pool)
                    v_ = tmp.tile([128, R2], BF16, tag="t1")
                    nc.vector.tensor_tensor(v_[:], oz_sb[:], n_sb[:],
                                            op=ALU.mult)
                    nc.vector.tensor_tensor(hsl(hnxt), o3(v_), o3(w_),
                                            op=ALU.add)
                    nc.gpsimd.tensor_tensor(hsl(h8nxt), o3(v_), o3(w_),
                                            op=ALU.add)

            def app(src8, dst, dst8, k):
                """One splitter application: t_k = s(t_{k-1})."""
                HR = R // 2
                for st in range(ARITY):
                    first = st == 0
                    hcur = ht[(st + 1) % 2]
                    h8cur = src8 if first else h8[(st + 1) % 2]
                    hnxt = dst if st == ARITY - 1 else ht[st % 2]
                    h8nxt = dst8 if st == ARITY - 1 else h8[st % 2]
                    for hf_ in range(2):
                        pg = {}
                        for gi_, g in enumerate("rzn"):
                            if first and g == "r":
                                continue
                            P = pool_box['p'].tile([128, 1024], FP32,
                                                   tag=f"h{hf_}")
                            for c in range(KC):
                                j = gi_ * 4 + c
                                Ph = P[:, c * HR:(c + 1) * HR]
                                if first:
                                    emit_gemm(Ph, w["ih_s"], j, src8, R,
                                              hf_=hf_)
                                elif g != "n":
                                    emit_gemm(Ph, w["ih_s"], j, src8, R,
                                              start=True, stop=False,
                                              hf_=hf_)
                                    emit_gemm(Ph, w["hh_s"], j, h8cur, R,
                                              start=False, stop=True,
                                              hf_=hf_)
                                else:
                                    emit_gemm(Ph, w["hh_s"], j, h8cur, R,
                                              hf_=hf_)
                            pg[g] = P
                        cell_chunk(hf_, pg.get("r"), pg["z"], pg["n"],
                                   hcur, hnxt, h8nxt, first)

            # ---------------- phase S+P ----------------
            if "sp" in PHASES:
                with tc.tile_pool(name="bigp", bufs=2, space="PSUM") as bigp:
                    pool_box['p'] = bigp
                    proj(x8, 0)
                    prev8 = x8
                    for k in range(1, DEPTH):
                        app(prev8, ts, t8, k)
                        pooled_k(ts, k)
                        proj(t8, k)
                        prev8 = t8
            elif "apponly" in PHASES:
                prev8 = x8
                for k in range(1, DEPTH + 1):
                    app(prev8, ts, t8, k)
                    pooled_k(ts, k)
                    prev8 = t8
            elif "projonly" in PHASES:
                for k in range(5):
                    proj(x8, k)

        # ---------------- scan ----------------
        if True:
            for c in "fb":
                nc.vector.memset(hs8[c][0][:], 0.0)

            def scan_step(chain, t):
                gt = gf if chain == "f" else gb
                s = t if chain == "f" else S - 1 - t
                cur8 = hs8[chain][t % 2]
                nxt8 = hs8[chain][1 - t % 2]
                pt = scp.tile([128, 512], FP32, tag=chain)
                rz = pt
                nn = pt
                wt = w[f"hh_{chain}"]
                # preload gi_rz via identity-matmul (start=True zeroes bank)
                nc.tensor.matmul(rz[:, 0:160], ident[:],
                                 gt[:, s * 240:s * 240 + 160],
                                 start=True, stop=False,
                                 skip_group_check=True)
                for j in range(8):
                    o = rz[:, j * W20:(j + 1) * W20]
                    if cfg.fp8:
                        for pr in range(2):
                            nc.tensor.matmul(
                                o, wap(wt, j, pr), mov(cur8, W20, pr),
                                start=False, stop=(pr == 1),
                                perf_mode=DR, skip_group_check=True)
                    else:
                        for cc in range(KC):
                            nc.tensor.matmul(
                                o, wt[:, cc * G3 + j * 128:
                                      cc * G3 + j * 128 + 128],
                                cur8[:, cc * W20:(cc + 1) * W20],
                                start=False, stop=(cc == KC - 1),
                                skip_group_check=True)
                for j in range(8, 12):
                    o = nn[:, 160 + (j - 8) * W20:160 + (j - 7) * W20]
                    if cfg.fp8:
                        for pr in range(2):
                            nc.tensor.matmul(
                                o, wap(wt, j, pr), mov(cur8, W20, pr),
                                start=(pr == 0), stop=(pr == 1),
                                perf_mode=DR, skip_group_check=True)
                    else:
                        for cc in range(KC):
                            nc.tensor.matmul(
                                o, wt[:, cc * G3 + j * 128:
                                      cc * G3 + j * 128 + 128],
                                cur8[:, cc * W20:(cc + 1) * W20],
                                start=(cc == 0), stop=(cc == KC - 1),
                                skip_group_check=True)
                rz_sb = stp.tile([128, 160], BF16, tag=f"r{chain}")
                nc.scalar.activation(rz_sb[:], rz[:, 0:160], AF.Sigmoid,
                                     scale=inv)
                t1 = stp.tile([128, 80], BF16, tag=f"t1{chain}")
                nc.vector.tensor_tensor(t1[:], rz_sb[:, 0:80],
                                        nn[:, 160:240], op=ALU.mult)
                t2 = stp.tile([128, 80], BF16, tag=f"t2{chain}")
                nc.vector.tensor_tensor(
                    t2[:], t1[:], gt[:, s * 240 + 160:s * 240 + 240],
                    op=ALU.add)
                n_sb = stp.tile([128, 80], BF16, tag=f"n{chain}")
                nc.scalar.activation(n_sb[:], t2[:], AF.Tanh, scale=inv)
                # off-path: m = oz*h, w = h - m (pool is dtype-blind)
                m_ = stp.tile([128, 80], BF16, tag=f"m{chain}")
                nc.gpsimd.tensor_tensor(m_[:], rz_sb[:, 80:160], cur8[:],
                                        op=ALU.mult)
                w_ = stp.tile([128, 80], BF16, tag=f"w{chain}")
                nc.gpsimd.tensor_tensor(w_[:], cur8[:], m_[:],
                                        op=ALU.subtract)
                # path: v = oz*n; h'8 = v + w
                v_ = stp.tile([128, 80], BF16, tag=f"v{chain}")
                nc.vector.tensor_tensor(v_[:], rz_sb[:, 80:160], n_sb[:],
                                        op=ALU.mult)
                nc.gpsimd.tensor_tensor(nxt8[:], v_[:], w_[:], op=ALU.add)

            if "scan" in PHASES:
                for t in range(S):
                    scan_step("f", t)
                    scan_step("b", t)
            if "sp" in PHASES:
                app(t8, ts, t8, DEPTH)
                pooled_k(ts, DEPTH)

            fin = S % 2
            nc.vector.tensor_copy(enc_sb[:, 0:80], hs8["f"][fin][:])
            nc.vector.tensor_copy(enc_sb[:, 80:160], hs8["b"][fin][:])
            nc.gpsimd.dma_start(enc_out[:, :], enc_sb[:])
            nc.gpsimd.dma_start(pooled_out[:, :], pooled_sb[:])
            app5_pool.__exit__(None, None, None)
            scan_pools.__exit__(None, None, None)

        for fr in reversed(frees):
            fr()
    return nc


# ---------------- host side ----------------

def _np_dt(dt):
    return ml_dtypes.float8_e4m3 if dt == FP8E4 else ml_dtypes.bfloat16


def make_inmaps(p, cfg: Cfg):
    x = np.asarray(p["x"], np.float32)
    ins = []

    def wprep(W):
        Wm = W.astype(np.float32) * cfg.scale
        Wm = Wm.copy()
        Wm[512:1024] *= -1.0  # negate z rows -> sigmoid gives 1-z
        arr = Wm.T.reshape(KC, 128, G3).transpose(1, 0, 2).reshape(
            128, KC * G3)
        return np.ascontiguousarray(arr).astype(_np_dt(cfg.w_dt))

    wmats = {}
    for m, pref in (("s", "ts"), ("f", "tgf"), ("b", "tgb")):
        for nm in ("bih", "bhh"):
            assert not np.any(np.asarray(p[f"{pref}_{nm}"])), \
                f"nonzero bias {pref}_{nm} not supported"
        wmats[f"wih_{m}"] = wprep(np.asarray(p[f"{pref}_Wih"]))
        wmats[f"whh_{m}"] = wprep(np.asarray(p[f"{pref}_Whh"]))
    ident = np.eye(128, dtype=ml_dtypes.bfloat16)

    for c in range(8):
        m = dict(wmats)
        m["ident"] = ident
        xl = x[4 * c:4 * c + 4]           # (4, S, H)
        arr = xl.transpose(2, 0, 1).reshape(KC, 128, NB * S)
        m["x8"] = np.ascontiguousarray(
            arr.transpose(1, 0, 2).reshape(128, KC * R)).astype(
                _np_dt(cfg.sh_dt))
        ins.append(m)
    return ins


def gather_out(results, p):
    EPS = 1e-10
    x = np.asarray(p["x"], np.float64)
    g = -np.log(-np.log(np.asarray(p["gumbel_u"], np.float64) + EPS) + EPS)
    lw = np.asarray(p["logits_W"], np.float64)
    lb = np.asarray(p["logits_b"], np.float64)
    dw = lw[1] - lw[0]
    db = lb[1] - lb[0]
    out_W = np.asarray(p["out_W"], np.float64)
    out_b = np.asarray(p["out_b"], np.float64)
    B = 32

    pooled = np.zeros((DEPTH + 1, B, H))
    pooled[0] = x.mean(1)
    margins = np.zeros((DEPTH, B))
    for c, r in enumerate(results):
        enc = np.asarray(r["enc_out"], np.float64)      # (128, 160)
        pp = np.asarray(r["pooled_out"], np.float64)    # (128, 80)
        hf = enc[:, 0:80]
        hb = enc[:, 80:160]
        for bl in range(NB):
            b = 4 * c + bl
            for d in range(DEPTH):
                vec = np.zeros(H)
                for ch in range(KC):
                    col = ch * 20 + d * 4 + bl
                    vec[ch * 128:(ch + 1) * 128] = hf[:, col] + hb[:, col]
                margins[d, b] = vec @ dw + db + g[d, b, 1] - g[d, b, 0]
            for k in range(1, DEPTH + 1):
                vec = np.zeros(H)
                for ch in range(KC):
                    col = (k - 1) * 16 + ch * 4 + bl
                    vec[ch * 128:(ch + 1) * 128] = pp[:, col]
                pooled[k, b] = vec / S

    out = np.zeros(256)
    for b in range(B):
        mb = 0
        for d in range(DEPTH):
            if margins[d, b] > 0:
                mb += 1
            else:
                break
        out += pooled[mb, b] @ out_W.T
    out += B * out_b
    return out.astype(np.float32)


_BUILT = {}
PREDICTED_NS = [None]


def _get_built(cfg=None):
    key = "k"
    if key not in _BUILT:
        if cfg is None:
            cfg = Cfg()
        nc = bass.Bass(trn_type="TRN2")
        build_kernel(nc, cfg)
        split_excess_waits(nc)
        PREDICTED_NS[0] = LAST_SIM_TIME[0]
        _BUILT[key] = (nc, cfg)
    return _BUILT[key]


def kernel(**inputs):
    from concourse import bass_utils
    inputs = {k: np.asarray(v) for k, v in inputs.items()}
    nc, cfg = _get_built()
    ins = make_inmaps(inputs, cfg)
    res = bass_utils.run_bass_kernel_spmd(nc, ins, core_ids=list(range(8)))
    return gather_out(res.results, inputs)


if __name__ == "__main__":
    import sys
    cfg = Cfg(fp8="nofp8" not in sys.argv)
    _get_built(cfg)
    print(f"HW exec time: {PREDICTED_NS[0]:.0f} ns")
